# revision 31
# baseline (speedup 1.0000x reference)
"""DIPNet soft-kNN kernel for 8 Trainium2 NeuronCores.

Structure (two SPMD launches, no collectives):

Launch A ("proj"): the 2048 tokens (4x256 target + 4x256 ref) are
data-parallel across 8 cores (256 tokens each). Each core runs
fc1 -> erf-GELU -> fc2 -> LayerNorm -> l2-normalize and returns its
(256, 384) block of normalized features (bf16).  Matmul operands are
bf16 (fp32 PSUM accumulate).  When the LayerNorm affine is a uniform
positive scale with zero shift and fc2's bias is uniform (true for this
model: w=1, b=0), LN + l2norm collapse exactly to
  out = (y - mean(y)) / sqrt(D * var(y))
(the rsqrt(var+eps) cancels under the l2 norm), which is one bn_stats
pass + one tensor_scalar pass straight out of PSUM.  A general variant
is built lazily if the params are ever non-uniform.

Between launches the host only moves data: transposes feature blocks,
builds per-patch class histograms from the int32 label map (a bincount),
and counts how often each batch b' appears among batch b's distractor
permutation blocks (c[b,b']).

Launch B ("attn"): core i handles (batch b = i//2, channel half
h = i%2).  Because the 8 distractor blocks are batch permutations,
attention over the 2048 keys collapses to the 4 distinct 256-key blocks
weighted by integer counts c[b,b']; ln(c) folds into the Exp bias, so
  PT[k, q] = c_k * exp(q . k / beta)        (ST matmul + one ACT op)
  pv[ch,q] = sum_k lab[k, ch] * PT[k, q]    (PV matmul, interleaved)
The softmax denominator is NOT computed on device: every key's label
histogram sums to exactly 1 over the full 1000 classes, so
Z[q] = sum_c pv_full[c, q], which the host gets for free when it
gathers the two channel halves.  The device ships raw pv (500, 256);
host divides by Z and nearest-upsamples 16x16 -> 224x224 during
unshard (the device writes 0.5 MB instead of 100 MB).
"""

import numpy as np

import concourse.bass as bass
import concourse.bacc as bacc
import concourse.mybir as mybir
import concourse.tile as tile
from concourse.tile import add_dep_helper
from concourse.bass_utils import run_bass_kernel_spmd

F32 = mybir.dt.float32
BF16 = mybir.dt.bfloat16
NP_BF16 = mybir.dt.np(BF16)
AF = mybir.ActivationFunctionType
ALU = mybir.AluOpType

BS, IMG, P14, GRID, NPATCH = 4, 224, 14, 16, 256
D, HID, NC, NDIST = 384, 2688, 1000, 7
BETA = 0.07
LN_EPS = 1e-6
NCORE = 8
TOK = 256           # tokens per core in launch A
KD = D // 128       # 3
KH = HID // 128     # 21
NK = 1024           # distinct keys (4 batches x 256 patches)
KK = NK // 128      # 8
CH = NC // 2        # 500 channels per core in launch B
CCH = 125           # channels per PV chunk (4 chunks)

# Results of the most recent launches (for test harnesses / profiling).
LAST_RESULTS = {}

_BUILT = {}


def _seal_dmas(nc, dma_insts):
    """Consume DMA completion sems with 1-wait SP nops so the kernel-tail
    drain's wait list stays under the walrus CTRL sync-wait limit."""
    for d in dma_insts:
        nop = nc.sync.nop().ins
        add_dep_helper(nop, d, True, "drain fanin")


def _build_proj(general):
    nc = bacc.Bacc("TRN2")
    # inputs are host-rearranged to partition-contiguous layouts so every
    # DMA lands as 128 large contiguous descriptors
    xT = nc.dram_tensor("xT", [128, KD, TOK], BF16, kind="ExternalInput")
    fc1 = nc.dram_tensor("fc1", [128, KH, KD, 128], BF16, kind="ExternalInput")
    b1 = nc.dram_tensor("b1", [128, KH], F32, kind="ExternalInput")
    fc2 = nc.dram_tensor("fc2", [128, KH, D], BF16, kind="ExternalInput")
    if general:
        b2 = nc.dram_tensor("b2", [D], F32, kind="ExternalInput")
        lnw = nc.dram_tensor("lnw", [D], F32, kind="ExternalInput")
        lnb = nc.dram_tensor("lnb", [D], F32, kind="ExternalInput")
    feat = nc.dram_tensor("feat", [128, TOK // 128, D], BF16,
                          kind="ExternalOutput")

    def bcast(t, n):
        a = t[:]
        return bass.AP(tensor=a.tensor, offset=a.offset, ap=[[0, 128], [1, n]])

    dmas = []
    with tile.TileContext(nc) as tc:
        with (
            tc.tile_pool(name="w", bufs=1) as wpool,
            tc.tile_pool(name="h", bufs=1) as hpool,
            tc.tile_pool(name="y", bufs=2) as ypool,
            tc.tile_pool(name="s", bufs=4) as spool,
            tc.tile_pool(name="ps1", bufs=3, space="PSUM") as ps1,
            tc.tile_pool(name="ps2", bufs=1, space="PSUM") as ps2,
            tc.tile_pool(name="psw", bufs=1, space="PSUM") as psw,
        ):
            # all weight chunks on the sync HWDGE queue IN CONSUMPTION ORDER
            # (one queue = strict priority; a second queue would round-robin
            # packets and starve the early fc1 chunks). xT/b1 ride gpsimd.
            fc1_sb = wpool.tile([128, KH, KD, 128], BF16)
            fc2_sb = wpool.tile([128, KH, D], BF16)
            # chunk sizes paced so each chunk lands before the matmul stream
            # finishes the previous one (DMA streams ~0.26 us/m-tile, fc1
            # consumes ~0.33, fc2 ~0.49; +2us completion latency per chunk)
            for lo, hi in [(0, 3), (3, 6), (6, 10), (10, 15), (15, 21)]:
                dmas.append(nc.sync.dma_start(
                    out=fc1_sb[:, lo:hi, :, :],
                    in_=fc1[:, lo:hi, :, :]).ins)
            for lo, hi in [(0, 4), (4, 9), (9, 15), (15, 21)]:
                dmas.append(nc.sync.dma_start(
                    out=fc2_sb[:, lo:hi, :],
                    in_=fc2[:, lo:hi, :]).ins)
            xT_sb = wpool.tile([128, KD, TOK], BF16)
            dmas.append(nc.gpsimd.dma_start(out=xT_sb, in_=xT[:, :, :]).ins)
            b1_sb = wpool.tile([128, KH], F32)
            dmas.append(nc.gpsimd.dma_start(out=b1_sb, in_=b1[:, :]).ins)
            # PE warm-up: dummy matmul stream during the DMA wait ramps the
            # tensor engine clock so the real stream starts at full speed
            ws = wpool.tile([128, TOK], BF16)
            nc.vector.memset(ws, 0.0)
            wps = psw.tile([128, TOK], F32)
            for _ in range(22):
                nc.tensor.matmul(wps, ws[:, :128], ws, start=True, stop=True)
            if general:
                b2_sb = wpool.tile([128, D], F32)
                dmas.append(nc.gpsimd.dma_start(out=b2_sb, in_=bcast(b2, D)).ins)
                lnw_sb = wpool.tile([128, D], F32)
                dmas.append(nc.gpsimd.dma_start(out=lnw_sb, in_=bcast(lnw, D)).ins)
                lnb_sb = wpool.tile([128, D], F32)
                dmas.append(nc.gpsimd.dma_start(out=lnb_sb, in_=bcast(lnb, D)).ins)
            eps_sb = wpool.tile([128, 1], F32)
            nc.vector.memset(eps_sb, LN_EPS if general else 1e-30)
            warm = wpool.tile([1, 1], F32)
            nc.vector.memset(warm, 1.0)
            nc.scalar.activation(warm, warm, AF.Gelu)
            nc.scalar.activation(warm, warm, AF.Sqrt)

            # fc1 + GELU -> hT (HID, TOK) laid out [128, KH, TOK]
            hT_sb = hpool.tile([128, KH, TOK], BF16)
            for m in range(KH):
                ps = ps1.tile([128, TOK], F32)
                for k in range(KD):
                    nc.tensor.matmul(
                        ps, fc1_sb[:, m, k, :], xT_sb[:, k, :],
                        start=(k == 0), stop=(k == KD - 1))
                nc.scalar.activation(hT_sb[:, m, :], ps, AF.Gelu,
                                     bias=b1_sb[:, m:m + 1])

            # fc2 -> y (TOK, D); m2-major so m2=0's LayerNorm overlaps
            # m2=1's matmuls
            for m2 in range(TOK // 128):
                ps = ps2.tile([128, D], F32, tag=f"ps2{m2}", name=f"ps2{m2}")
                for k in range(KH):
                    nc.tensor.matmul(
                        ps, hT_sb[:, k, m2 * 128:(m2 + 1) * 128],
                        fc2_sb[:, k, :],
                        start=(k == 0), stop=(k == KH - 1))
                st = spool.tile([128, 6], F32, tag="st")
                mv = spool.tile([128, 2], F32, tag="mv")
                ym = ypool.tile([128, D], BF16, tag="ym")
                if not general:
                    # l2norm(LN(y)) == (y - mu) / sqrt(D * var): the
                    # rsqrt(var+eps) cancels under the l2 norm when the LN
                    # affine is a uniform positive scale with zero shift and
                    # fc2's bias is uniform.
                    nc.vector.bn_stats(out=st, in_=ps)
                    nc.vector.bn_aggr(out=mv, in_=st)
                    sd = spool.tile([128, 1], F32, tag="sd")
                    nc.scalar.activation(sd, mv[:, 1:2], AF.Sqrt,
                                         bias=eps_sb, scale=float(D))
                    rn = spool.tile([128, 1], F32, tag="rn")
                    nc.vector.reciprocal(rn, sd)
                    nc.vector.tensor_scalar(ym, ps, mv[:, 0:1], rn,
                                            op0=ALU.subtract, op1=ALU.mult)
                else:
                    y = ypool.tile([128, D], F32, tag="y")
                    nc.vector.tensor_add(y, ps, b2_sb)
                    nc.vector.bn_stats(out=st, in_=y)
                    nc.vector.bn_aggr(out=mv, in_=st)
                    sd = spool.tile([128, 1], F32, tag="sd")
                    nc.scalar.activation(sd, mv[:, 1:2], AF.Sqrt, bias=eps_sb)
                    rstd = spool.tile([128, 1], F32, tag="rstd")
                    nc.vector.reciprocal(rstd, sd)
                    nc.vector.tensor_scalar(y, y, mv[:, 0:1], rstd,
                                            op0=ALU.subtract, op1=ALU.mult)
                    nc.vector.tensor_mul(y, y, lnw_sb)
                    nc.vector.tensor_add(y, y, lnb_sb)
                    sq = ypool.tile([128, D], F32, tag="sq")
                    ssq = spool.tile([128, 1], F32, tag="ssq")
                    nc.scalar.activation(sq, y, AF.Square, accum_out=ssq)
                    nrm = spool.tile([128, 1], F32, tag="nrm")
                    nc.scalar.activation(nrm, ssq, AF.Sqrt)
                    rn = spool.tile([128, 1], F32, tag="rn")
                    nc.vector.reciprocal(rn, nrm)
                    nc.vector.tensor_scalar_mul(ym, y, rn)
                dmas.append(nc.sync.dma_start(out=feat[:, m2, :], in_=ym).ins)

            _seal_dmas(nc, dmas)
    nc.compile()
    return nc


def _build_attn():
    nc = bacc.Bacc("TRN2")
    qT = nc.dram_tensor("qT", [128, KD, NPATCH], BF16, kind="ExternalInput")
    kT = nc.dram_tensor("kT", [128, KD, NK], BF16, kind="ExternalInput")
    # lab padded to 512 channels so PV weights are 128 wide (FWL-eligible)
    lab = nc.dram_tensor("lab", [128, KK, 512], BF16, kind="ExternalInput")
    lnc = nc.dram_tensor("lnc", [KK], F32, kind="ExternalInput")
    # raw pv, packed [128, 4, 256]: out[p, ci, q] = pv[ci*128+p, q].
    # bf16: the host divides pv/Z, so the ~0.4% rounding largely cancels.
    out = nc.dram_tensor("out", [128, 4, NPATCH], BF16, kind="ExternalOutput")

    dmas = []
    with tile.TileContext(nc) as tc:
        with (
            tc.tile_pool(name="w", bufs=1) as wpool,
            tc.tile_pool(name="pt", bufs=3) as ptpool,
            tc.tile_pool(name="lh", bufs=1) as lhpool,
            tc.tile_pool(name="ps", bufs=2, space="PSUM") as psp,
            tc.tile_pool(name="pv", bufs=1, space="PSUM") as pvp,
            tc.tile_pool(name="psw", bufs=1, space="PSUM") as psw,
        ):
            # ONE sync HWDGE queue, in exact consumption order: a second
            # parallel queue would round-robin packets and delay kT c0 (the
            # first-ST gate) behind lab bytes it doesn't need yet.
            qT_sb = wpool.tile([128, KD, NPATCH], BF16)
            kT_sb = wpool.tile([128, KD, NK], BF16)
            lab_sb = wpool.tile([128, KK, 512], BF16)
            dmas.append(nc.sync.dma_start(out=qT_sb, in_=qT[:, :, :]).ins)
            for c in range(4):
                dmas.append(nc.sync.dma_start(
                    out=kT_sb[:, :, c * 256:(c + 1) * 256],
                    in_=kT[:, :, c * 256:(c + 1) * 256]).ins)
                dmas.append(nc.sync.dma_start(
                    out=lab_sb[:, c * 2:(c + 1) * 2, :],
                    in_=lab[:, c * 2:(c + 1) * 2, :]).ins)
            lnc_ap = lnc[:]
            lnc_sb = wpool.tile([128, KK], F32)
            dmas.append(nc.gpsimd.dma_start(
                out=lnc_sb,
                in_=bass.AP(tensor=lnc_ap.tensor, offset=lnc_ap.offset,
                            ap=[[0, 128], [1, KK]])).ins)
            warm = wpool.tile([1, 1], F32)
            nc.vector.memset(warm, 1.0)
            nc.scalar.activation(warm, warm, AF.Exp)
            # PE warm-up stream (ramps the clock during the DMA wait)
            ws = wpool.tile([128, NPATCH], BF16)
            nc.vector.memset(ws, 0.0)
            wps = psw.tile([128, NPATCH], F32)
            for _ in range(22):
                nc.tensor.matmul(wps, ws[:, :128], ws, start=True, stop=True)

            # PT[k, q] = c_k * exp((q.k)/beta); software-pipelined: emit
            # ST(km+1) before PV(km) so the tensor engine computes the next
            # score block while the scalar engine runs Exp(km).
            pv = [pvp.tile([128, NPATCH], F32, tag=f"pv{ci}", name=f"pv{ci}")
                  for ci in range(4)]
            pts = []

            def st_block(km):
                ps = psp.tile([128, NPATCH], F32, tag="st", name="ps")
                for k in range(KD):
                    nc.tensor.matmul(
                        ps, kT_sb[:, k, km * 128:(km + 1) * 128], qT_sb[:, k, :],
                        start=(k == 0), stop=(k == KD - 1))
                pt = ptpool.tile([128, NPATCH], BF16, tag="pt", name="pt")
                nc.scalar.activation(pt, ps, AF.Exp,
                                     bias=lnc_sb[:, km:km + 1], scale=1.0 / BETA)
                pts.append(pt)

            def pv_block(km):
                for ci in range(4):
                    nc.tensor.matmul(
                        pv[ci], lab_sb[:, km, ci * 128:(ci + 1) * 128], pts[km],
                        start=(km == 0), stop=(km == KK - 1))

            st_block(0)
            for km in range(1, KK):
                st_block(km)
                pv_block(km - 1)
            pv_block(KK - 1)

            # pack the 4 raw pv chunks into one SBUF tile, single DMA out
            big = lhpool.tile([128, 4, NPATCH], BF16)
            for ci in range(4):
                if ci % 2 == 0:
                    nc.vector.tensor_copy(big[:, ci, :], pv[ci])
                else:
                    nc.scalar.copy(big[:, ci, :], pv[ci])
            dmas.append(nc.sync.dma_start(out=out[:, :, :], in_=big).ins)

            _seal_dmas(nc, dmas)
    nc.compile()
    return nc


def _get(name):
    if name not in _BUILT:
        if name == "proj_fast":
            _BUILT[name] = _build_proj(False)
        elif name == "proj_gen":
            _BUILT[name] = _build_proj(True)
        else:
            _BUILT[name] = _build_attn()
    return _BUILT[name]


def kernel(feat_target, feat_ref, labels_ref, perms, fc1_w, fc1_b, fc2_w,
           fc2_b, ln_w, ln_b):
    f32 = lambda a: np.ascontiguousarray(a, dtype=np.float32)
    bf16 = lambda a: np.ascontiguousarray(np.asarray(a, dtype=np.float32)
                                          .astype(NP_BF16))

    ln_w = np.asarray(ln_w, dtype=np.float32)
    ln_b = np.asarray(ln_b, dtype=np.float32)
    fc2_b_np = np.asarray(fc2_b, dtype=np.float32)
    fast = (np.all(ln_w == ln_w.flat[0]) and ln_w.flat[0] > 0
            and np.all(ln_b == 0.0) and np.all(fc2_b_np == fc2_b_np.flat[0]))

    # ---- Launch A: projection, data-parallel over the 2048 tokens ----
    X = np.concatenate([np.asarray(feat_target).reshape(BS * NPATCH, D),
                        np.asarray(feat_ref).reshape(BS * NPATCH, D)], axis=0)
    fc1_pc = bf16(np.asarray(fc1_w).reshape(KD, 128, KH, 128)
                  .transpose(1, 2, 0, 3))               # (128, KH, KD, 128)
    fc2_pc = bf16(np.asarray(fc2_w).reshape(KH, 128, D).transpose(1, 0, 2))
    b1_pc = f32(np.asarray(fc1_b).reshape(KH, 128).T)
    shared = {"fc1": fc1_pc, "b1": b1_pc, "fc2": fc2_pc}
    if not fast:
        shared.update({"b2": f32(fc2_b_np), "lnw": f32(ln_w), "lnb": f32(ln_b)})

    def pc(mat_t):     # (D, n) -> (128, KD, n) partition-contiguous
        return bf16(mat_t.reshape(-1, 128, mat_t.shape[-1]).transpose(1, 0, 2))

    in_maps = [{"xT": pc(X[i * TOK:(i + 1) * TOK].T), **shared}
               for i in range(NCORE)]
    res_a = run_bass_kernel_spmd(_get("proj_fast" if fast else "proj_gen"),
                                 in_maps, core_ids=list(range(NCORE)))
    LAST_RESULTS["proj"] = res_a
    # feat block i is [128, 2, 384]: token t = m2*128 + p  ->  row p, slot m2
    F = np.concatenate(
        [np.asarray(res_a.results[i]["feat"]).transpose(1, 0, 2)
         .reshape(TOK, D) for i in range(NCORE)], axis=0)
    qf = F[:BS * NPATCH].reshape(BS, NPATCH, D)
    kTf = pc(np.ascontiguousarray(F[BS * NPATCH:].T))   # (128, KD, 1024)

    # ---- host: label histograms + distractor-permutation counts ----
    g = np.asarray(labels_ref).reshape(BS, GRID, P14, GRID, P14)
    g = g.transpose(0, 1, 3, 2, 4).reshape(BS, NPATCH, P14 * P14)
    ids = (np.arange(NPATCH, dtype=np.int64)[None, :, None] * NC + g).reshape(BS, -1)
    label = np.stack([np.bincount(ids[b], minlength=NPATCH * NC)
                      for b in range(BS)]).astype(np.float32)
    label = label.reshape(BS * NPATCH, NC) / float(P14 * P14)   # (1024, 1000)

    all_perm = np.concatenate([np.arange(BS, dtype=np.int64)[None, :],
                               np.asarray(perms).astype(np.int64)], axis=0)
    cnt = np.zeros((BS, BS), dtype=np.float64)
    for j in range(NDIST + 1):
        for b in range(BS):
            cnt[b, all_perm[j, b]] += 1.0

    label_pad = np.zeros((BS * NPATCH, 2 * 512), dtype=np.float32)
    label_pad[:, :CH] = label[:, :CH]
    label_pad[:, 512:512 + CH] = label[:, CH:]

    # ---- Launch B: attention, core i = (batch i//2, half i%2) ----
    in_maps_b = []
    for i in range(NCORE):
        b, h = i // 2, i % 2
        lnc8 = np.where(cnt[b, np.arange(KK) // 2] > 0,
                        np.log(np.maximum(cnt[b, np.arange(KK) // 2], 1e-30)),
                        -1e30).astype(np.float32)
        in_maps_b.append({
            "qT": pc(np.ascontiguousarray(qf[b].T)),
            "kT": kTf,
            "lab": bf16(label_pad[:, h * 512:(h + 1) * 512]
                        .reshape(KK, 128, 512).transpose(1, 0, 2)),
            "lnc": lnc8,
        })
    res_b = run_bass_kernel_spmd(_get("attn"), in_maps_b,
                                 core_ids=list(range(NCORE)))
    LAST_RESULTS["attn"] = res_b

    # ---- host unshard: gather raw pv, softmax-normalize, upsample ----
    pv = np.empty((BS, NC, NPATCH), dtype=np.float32)
    for i in range(NCORE):
        b, h = i // 2, i % 2
        # out[p, ci, q] -> rows ci*128+p of this (padded) half
        blk = np.asarray(res_b.results[i]["out"]).astype(np.float32)
        blk = blk.transpose(1, 0, 2)
        pv[b, h * CH:(h + 1) * CH] = blk.reshape(512, NPATCH)[:CH]
    z = pv.sum(axis=1, keepdims=True)           # sum_c lab = 1 per key
    lab_hat = (pv / z).reshape(BS, NC, GRID, GRID)
    out = np.broadcast_to(
        lab_hat[:, :, :, None, :, None],
        (BS, NC, GRID, P14, GRID, P14)).reshape(BS, NC, IMG, IMG)
    return out


# revision 32
# speedup vs baseline: 1.0026x; 1.0026x over previous
"""DIPNet soft-kNN kernel for 8 Trainium2 NeuronCores.

Structure (two SPMD launches, no collectives):

Launch A ("proj"): the 2048 tokens (4x256 target + 4x256 ref) are
data-parallel across 8 cores (256 tokens each). Each core runs
fc1 -> erf-GELU -> fc2 -> LayerNorm -> l2-normalize and returns its
(256, 384) block of normalized features (bf16).  Matmul operands are
bf16 (fp32 PSUM accumulate).  When the LayerNorm affine is a uniform
positive scale with zero shift and fc2's bias is uniform (true for this
model: w=1, b=0), LN + l2norm collapse exactly to
  out = (y - mean(y)) / sqrt(D * var(y))
(the rsqrt(var+eps) cancels under the l2 norm), which is one bn_stats
pass + one tensor_scalar pass straight out of PSUM.  A general variant
is built lazily if the params are ever non-uniform.

Between launches the host only moves data: transposes feature blocks,
builds per-patch class histograms from the int32 label map (a bincount),
and counts how often each batch b' appears among batch b's distractor
permutation blocks (c[b,b']).

Launch B ("attn"): core i handles (batch b = i//2, channel half
h = i%2).  Because the 8 distractor blocks are batch permutations,
attention over the 2048 keys collapses to the 4 distinct 256-key blocks
weighted by integer counts c[b,b']; ln(c) folds into the Exp bias, so
  PT[k, q] = c_k * exp(q . k / beta)        (ST matmul + one ACT op)
  pv[ch,q] = sum_k lab[k, ch] * PT[k, q]    (PV matmul, interleaved)
The softmax denominator is NOT computed on device: every key's label
histogram sums to exactly 1 over the full 1000 classes, so
Z[q] = sum_c pv_full[c, q], which the host gets for free when it
gathers the two channel halves.  The device ships raw pv (500, 256);
host divides by Z and nearest-upsamples 16x16 -> 224x224 during
unshard (the device writes 0.5 MB instead of 100 MB).
"""

import numpy as np

import concourse.bass as bass
import concourse.bacc as bacc
import concourse.mybir as mybir
import concourse.tile as tile
from concourse.tile import add_dep_helper
from concourse.bass_utils import run_bass_kernel_spmd

F32 = mybir.dt.float32
BF16 = mybir.dt.bfloat16
NP_BF16 = mybir.dt.np(BF16)
AF = mybir.ActivationFunctionType
ALU = mybir.AluOpType

BS, IMG, P14, GRID, NPATCH = 4, 224, 14, 16, 256
D, HID, NC, NDIST = 384, 2688, 1000, 7
BETA = 0.07
LN_EPS = 1e-6
NCORE = 8
TOK = 256           # tokens per core in launch A
KD = D // 128       # 3
KH = HID // 128     # 21
NK = 1024           # distinct keys (4 batches x 256 patches)
KK = NK // 128      # 8
CH = NC // 2        # 500 channels per core in launch B
CCH = 125           # channels per PV chunk (4 chunks)

# Results of the most recent launches (for test harnesses / profiling).
LAST_RESULTS = {}

_BUILT = {}


def _seal_dmas(nc, dma_insts):
    """Consume DMA completion sems with 1-wait SP nops so the kernel-tail
    drain's wait list stays under the walrus CTRL sync-wait limit."""
    for d in dma_insts:
        nop = nc.sync.nop().ins
        add_dep_helper(nop, d, True, "drain fanin")


def _build_proj(general):
    nc = bacc.Bacc("TRN2")
    # inputs are host-rearranged to partition-contiguous layouts so every
    # DMA lands as 128 large contiguous descriptors
    xT = nc.dram_tensor("xT", [128, KD, TOK], BF16, kind="ExternalInput")
    fc1 = nc.dram_tensor("fc1", [128, KH, KD, 128], BF16, kind="ExternalInput")
    b1 = nc.dram_tensor("b1", [128, KH], F32, kind="ExternalInput")
    fc2 = nc.dram_tensor("fc2", [128, KH, D], BF16, kind="ExternalInput")
    if general:
        b2 = nc.dram_tensor("b2", [D], F32, kind="ExternalInput")
        lnw = nc.dram_tensor("lnw", [D], F32, kind="ExternalInput")
        lnb = nc.dram_tensor("lnb", [D], F32, kind="ExternalInput")
    feat = nc.dram_tensor("feat", [128, TOK // 128, D], BF16,
                          kind="ExternalOutput")

    def bcast(t, n):
        a = t[:]
        return bass.AP(tensor=a.tensor, offset=a.offset, ap=[[0, 128], [1, n]])

    dmas = []
    with tile.TileContext(nc) as tc:
        with (
            tc.tile_pool(name="w", bufs=1) as wpool,
            tc.tile_pool(name="h", bufs=1) as hpool,
            tc.tile_pool(name="y", bufs=2) as ypool,
            tc.tile_pool(name="s", bufs=4) as spool,
            tc.tile_pool(name="ps1", bufs=3, space="PSUM") as ps1,
            tc.tile_pool(name="ps2", bufs=1, space="PSUM") as ps2,
            tc.tile_pool(name="psw", bufs=1, space="PSUM") as psw,
        ):
            # all weight chunks on the sync HWDGE queue IN CONSUMPTION ORDER
            # (one queue = strict priority; a second queue would round-robin
            # packets and starve the early fc1 chunks). xT/b1 ride gpsimd.
            fc1_sb = wpool.tile([128, KH, KD, 128], BF16)
            fc2_sb = wpool.tile([128, KH, D], BF16)
            # chunk sizes paced so each chunk lands before the matmul stream
            # finishes the previous one (DMA streams ~0.26 us/m-tile, fc1
            # consumes ~0.33, fc2 ~0.49; +2us completion latency per chunk)
            for lo, hi in [(0, 3), (3, 6), (6, 10), (10, 15), (15, 21)]:
                dmas.append(nc.sync.dma_start(
                    out=fc1_sb[:, lo:hi, :, :],
                    in_=fc1[:, lo:hi, :, :]).ins)
            for lo, hi in [(0, 4), (4, 9), (9, 15), (15, 21)]:
                dmas.append(nc.sync.dma_start(
                    out=fc2_sb[:, lo:hi, :],
                    in_=fc2[:, lo:hi, :]).ins)
            xT_sb = wpool.tile([128, KD, TOK], BF16)
            dmas.append(nc.gpsimd.dma_start(out=xT_sb, in_=xT[:, :, :]).ins)
            b1_sb = wpool.tile([128, KH], F32)
            dmas.append(nc.gpsimd.dma_start(out=b1_sb, in_=b1[:, :]).ins)
            # PE warm-up: dummy matmul stream during the DMA wait ramps the
            # tensor engine clock so the real stream starts at full speed
            ws = wpool.tile([128, TOK], BF16)
            nc.vector.memset(ws, 0.0)
            wps = psw.tile([128, TOK], F32)
            for _ in range(22):
                nc.tensor.matmul(wps, ws[:, :128], ws, start=True, stop=True)
            if general:
                b2_sb = wpool.tile([128, D], F32)
                dmas.append(nc.gpsimd.dma_start(out=b2_sb, in_=bcast(b2, D)).ins)
                lnw_sb = wpool.tile([128, D], F32)
                dmas.append(nc.gpsimd.dma_start(out=lnw_sb, in_=bcast(lnw, D)).ins)
                lnb_sb = wpool.tile([128, D], F32)
                dmas.append(nc.gpsimd.dma_start(out=lnb_sb, in_=bcast(lnb, D)).ins)
            eps_sb = wpool.tile([128, 1], F32)
            nc.vector.memset(eps_sb, LN_EPS if general else 1e-30)
            warm = wpool.tile([1, 1], F32)
            nc.vector.memset(warm, 1.0)
            nc.scalar.activation(warm, warm, AF.Gelu)
            nc.scalar.activation(warm, warm, AF.Sqrt)

            # fc1 + GELU -> hT (HID, TOK) laid out [128, KH, TOK]
            hT_sb = hpool.tile([128, KH, TOK], BF16)
            for m in range(KH):
                ps = ps1.tile([128, TOK], F32)
                for k in range(KD):
                    nc.tensor.matmul(
                        ps, fc1_sb[:, m, k, :], xT_sb[:, k, :],
                        start=(k == 0), stop=(k == KD - 1))
                nc.scalar.activation(hT_sb[:, m, :], ps, AF.Gelu,
                                     bias=b1_sb[:, m:m + 1])

            # fc2 -> y (TOK, D); m2-major so m2=0's LayerNorm overlaps
            # m2=1's matmuls
            for m2 in range(TOK // 128):
                ps = ps2.tile([128, D], F32, tag=f"ps2{m2}", name=f"ps2{m2}")
                for k in range(KH):
                    nc.tensor.matmul(
                        ps, hT_sb[:, k, m2 * 128:(m2 + 1) * 128],
                        fc2_sb[:, k, :],
                        start=(k == 0), stop=(k == KH - 1))
                st = spool.tile([128, 6], F32, tag="st")
                mv = spool.tile([128, 2], F32, tag="mv")
                ym = ypool.tile([128, D], BF16, tag="ym")
                if not general:
                    # l2norm(LN(y)) == (y - mu) / sqrt(D * var): the
                    # rsqrt(var+eps) cancels under the l2 norm when the LN
                    # affine is a uniform positive scale with zero shift and
                    # fc2's bias is uniform.
                    nc.vector.bn_stats(out=st, in_=ps)
                    nc.vector.bn_aggr(out=mv, in_=st)
                    sd = spool.tile([128, 1], F32, tag="sd")
                    nc.scalar.activation(sd, mv[:, 1:2], AF.Sqrt,
                                         bias=eps_sb, scale=float(D))
                    rn = spool.tile([128, 1], F32, tag="rn")
                    nc.vector.reciprocal(rn, sd)
                    nc.vector.tensor_scalar(ym, ps, mv[:, 0:1], rn,
                                            op0=ALU.subtract, op1=ALU.mult)
                else:
                    y = ypool.tile([128, D], F32, tag="y")
                    nc.vector.tensor_add(y, ps, b2_sb)
                    nc.vector.bn_stats(out=st, in_=y)
                    nc.vector.bn_aggr(out=mv, in_=st)
                    sd = spool.tile([128, 1], F32, tag="sd")
                    nc.scalar.activation(sd, mv[:, 1:2], AF.Sqrt, bias=eps_sb)
                    rstd = spool.tile([128, 1], F32, tag="rstd")
                    nc.vector.reciprocal(rstd, sd)
                    nc.vector.tensor_scalar(y, y, mv[:, 0:1], rstd,
                                            op0=ALU.subtract, op1=ALU.mult)
                    nc.vector.tensor_mul(y, y, lnw_sb)
                    nc.vector.tensor_add(y, y, lnb_sb)
                    sq = ypool.tile([128, D], F32, tag="sq")
                    ssq = spool.tile([128, 1], F32, tag="ssq")
                    nc.scalar.activation(sq, y, AF.Square, accum_out=ssq)
                    nrm = spool.tile([128, 1], F32, tag="nrm")
                    nc.scalar.activation(nrm, ssq, AF.Sqrt)
                    rn = spool.tile([128, 1], F32, tag="rn")
                    nc.vector.reciprocal(rn, nrm)
                    nc.vector.tensor_scalar_mul(ym, y, rn)
                dmas.append(nc.sync.dma_start(out=feat[:, m2, :], in_=ym).ins)

            _seal_dmas(nc, dmas)
    nc.compile()
    return nc


def _build_attn():
    nc = bacc.Bacc("TRN2")
    qT = nc.dram_tensor("qT", [128, KD, NPATCH], BF16, kind="ExternalInput")
    kT = nc.dram_tensor("kT", [128, KD, NK], BF16, kind="ExternalInput")
    # lab padded to 512 channels so PV weights are 128 wide (FWL-eligible)
    lab = nc.dram_tensor("lab", [128, KK, 512], BF16, kind="ExternalInput")
    lnc = nc.dram_tensor("lnc", [KK], F32, kind="ExternalInput")
    # raw pv, packed [128, 4, 256]: out[p, ci, q] = pv[ci*128+p, q]
    out = nc.dram_tensor("out", [128, 4, NPATCH], F32, kind="ExternalOutput")

    dmas = []
    with tile.TileContext(nc) as tc:
        with (
            tc.tile_pool(name="w", bufs=1) as wpool,
            tc.tile_pool(name="pt", bufs=3) as ptpool,
            tc.tile_pool(name="lh", bufs=1) as lhpool,
            tc.tile_pool(name="ps", bufs=2, space="PSUM") as psp,
            tc.tile_pool(name="pv", bufs=1, space="PSUM") as pvp,
            tc.tile_pool(name="psw", bufs=1, space="PSUM") as psw,
        ):
            # ONE sync HWDGE queue, in exact consumption order: a second
            # parallel queue would round-robin packets and delay kT c0 (the
            # first-ST gate) behind lab bytes it doesn't need yet.
            qT_sb = wpool.tile([128, KD, NPATCH], BF16)
            kT_sb = wpool.tile([128, KD, NK], BF16)
            lab_sb = wpool.tile([128, KK, 512], BF16)
            dmas.append(nc.sync.dma_start(out=qT_sb, in_=qT[:, :, :]).ins)
            for c in range(4):
                dmas.append(nc.sync.dma_start(
                    out=kT_sb[:, :, c * 256:(c + 1) * 256],
                    in_=kT[:, :, c * 256:(c + 1) * 256]).ins)
                dmas.append(nc.sync.dma_start(
                    out=lab_sb[:, c * 2:(c + 1) * 2, :],
                    in_=lab[:, c * 2:(c + 1) * 2, :]).ins)
            lnc_ap = lnc[:]
            lnc_sb = wpool.tile([128, KK], F32)
            dmas.append(nc.gpsimd.dma_start(
                out=lnc_sb,
                in_=bass.AP(tensor=lnc_ap.tensor, offset=lnc_ap.offset,
                            ap=[[0, 128], [1, KK]])).ins)
            warm = wpool.tile([1, 1], F32)
            nc.vector.memset(warm, 1.0)
            nc.scalar.activation(warm, warm, AF.Exp)
            # PE warm-up stream (ramps the clock during the DMA wait)
            ws = wpool.tile([128, NPATCH], BF16)
            nc.vector.memset(ws, 0.0)
            wps = psw.tile([128, NPATCH], F32)
            for _ in range(22):
                nc.tensor.matmul(wps, ws[:, :128], ws, start=True, stop=True)

            # PT[k, q] = c_k * exp((q.k)/beta); software-pipelined: emit
            # ST(km+1) before PV(km) so the tensor engine computes the next
            # score block while the scalar engine runs Exp(km).
            pv = [pvp.tile([128, NPATCH], F32, tag=f"pv{ci}", name=f"pv{ci}")
                  for ci in range(4)]
            pts = []

            def st_block(km):
                ps = psp.tile([128, NPATCH], F32, tag="st", name="ps")
                for k in range(KD):
                    nc.tensor.matmul(
                        ps, kT_sb[:, k, km * 128:(km + 1) * 128], qT_sb[:, k, :],
                        start=(k == 0), stop=(k == KD - 1))
                pt = ptpool.tile([128, NPATCH], BF16, tag="pt", name="pt")
                nc.scalar.activation(pt, ps, AF.Exp,
                                     bias=lnc_sb[:, km:km + 1], scale=1.0 / BETA)
                pts.append(pt)

            def pv_block(km):
                for ci in range(4):
                    nc.tensor.matmul(
                        pv[ci], lab_sb[:, km, ci * 128:(ci + 1) * 128], pts[km],
                        start=(km == 0), stop=(km == KK - 1))

            st_block(0)
            for km in range(1, KK):
                st_block(km)
                pv_block(km - 1)
            pv_block(KK - 1)

            # pack the 4 raw pv chunks into one SBUF tile, single DMA out
            big = lhpool.tile([128, 4, NPATCH], F32)
            for ci in range(4):
                if ci % 2 == 0:
                    nc.vector.tensor_copy(big[:, ci, :], pv[ci])
                else:
                    nc.scalar.copy(big[:, ci, :], pv[ci])
            dmas.append(nc.sync.dma_start(out=out[:, :, :], in_=big).ins)

            _seal_dmas(nc, dmas)
    nc.compile()
    return nc


def _get(name):
    if name not in _BUILT:
        if name == "proj_fast":
            _BUILT[name] = _build_proj(False)
        elif name == "proj_gen":
            _BUILT[name] = _build_proj(True)
        else:
            _BUILT[name] = _build_attn()
    return _BUILT[name]


def kernel(feat_target, feat_ref, labels_ref, perms, fc1_w, fc1_b, fc2_w,
           fc2_b, ln_w, ln_b):
    f32 = lambda a: np.ascontiguousarray(a, dtype=np.float32)
    bf16 = lambda a: np.ascontiguousarray(np.asarray(a, dtype=np.float32)
                                          .astype(NP_BF16))

    ln_w = np.asarray(ln_w, dtype=np.float32)
    ln_b = np.asarray(ln_b, dtype=np.float32)
    fc2_b_np = np.asarray(fc2_b, dtype=np.float32)
    fast = (np.all(ln_w == ln_w.flat[0]) and ln_w.flat[0] > 0
            and np.all(ln_b == 0.0) and np.all(fc2_b_np == fc2_b_np.flat[0]))

    # ---- Launch A: projection, data-parallel over the 2048 tokens ----
    X = np.concatenate([np.asarray(feat_target).reshape(BS * NPATCH, D),
                        np.asarray(feat_ref).reshape(BS * NPATCH, D)], axis=0)
    fc1_pc = bf16(np.asarray(fc1_w).reshape(KD, 128, KH, 128)
                  .transpose(1, 2, 0, 3))               # (128, KH, KD, 128)
    fc2_pc = bf16(np.asarray(fc2_w).reshape(KH, 128, D).transpose(1, 0, 2))
    b1_pc = f32(np.asarray(fc1_b).reshape(KH, 128).T)
    shared = {"fc1": fc1_pc, "b1": b1_pc, "fc2": fc2_pc}
    if not fast:
        shared.update({"b2": f32(fc2_b_np), "lnw": f32(ln_w), "lnb": f32(ln_b)})

    def pc(mat_t):     # (D, n) -> (128, KD, n) partition-contiguous
        return bf16(mat_t.reshape(-1, 128, mat_t.shape[-1]).transpose(1, 0, 2))

    in_maps = [{"xT": pc(X[i * TOK:(i + 1) * TOK].T), **shared}
               for i in range(NCORE)]
    res_a = run_bass_kernel_spmd(_get("proj_fast" if fast else "proj_gen"),
                                 in_maps, core_ids=list(range(NCORE)))
    LAST_RESULTS["proj"] = res_a
    # feat block i is [128, 2, 384]: token t = m2*128 + p  ->  row p, slot m2
    F = np.concatenate(
        [np.asarray(res_a.results[i]["feat"]).transpose(1, 0, 2)
         .reshape(TOK, D) for i in range(NCORE)], axis=0)
    qf = F[:BS * NPATCH].reshape(BS, NPATCH, D)
    kTf = pc(np.ascontiguousarray(F[BS * NPATCH:].T))   # (128, KD, 1024)

    # ---- host: label histograms + distractor-permutation counts ----
    g = np.asarray(labels_ref).reshape(BS, GRID, P14, GRID, P14)
    g = g.transpose(0, 1, 3, 2, 4).reshape(BS, NPATCH, P14 * P14)
    ids = (np.arange(NPATCH, dtype=np.int64)[None, :, None] * NC + g).reshape(BS, -1)
    label = np.stack([np.bincount(ids[b], minlength=NPATCH * NC)
                      for b in range(BS)]).astype(np.float32)
    label = label.reshape(BS * NPATCH, NC) / float(P14 * P14)   # (1024, 1000)

    all_perm = np.concatenate([np.arange(BS, dtype=np.int64)[None, :],
                               np.asarray(perms).astype(np.int64)], axis=0)
    cnt = np.zeros((BS, BS), dtype=np.float64)
    for j in range(NDIST + 1):
        for b in range(BS):
            cnt[b, all_perm[j, b]] += 1.0

    label_pad = np.zeros((BS * NPATCH, 2 * 512), dtype=np.float32)
    label_pad[:, :CH] = label[:, :CH]
    label_pad[:, 512:512 + CH] = label[:, CH:]

    # ---- Launch B: attention, core i = (batch i//2, half i%2) ----
    in_maps_b = []
    for i in range(NCORE):
        b, h = i // 2, i % 2
        lnc8 = np.where(cnt[b, np.arange(KK) // 2] > 0,
                        np.log(np.maximum(cnt[b, np.arange(KK) // 2], 1e-30)),
                        -1e30).astype(np.float32)
        in_maps_b.append({
            "qT": pc(np.ascontiguousarray(qf[b].T)),
            "kT": kTf,
            "lab": bf16(label_pad[:, h * 512:(h + 1) * 512]
                        .reshape(KK, 128, 512).transpose(1, 0, 2)),
            "lnc": lnc8,
        })
    res_b = run_bass_kernel_spmd(_get("attn"), in_maps_b,
                                 core_ids=list(range(NCORE)))
    LAST_RESULTS["attn"] = res_b

    # ---- host unshard: gather raw pv, softmax-normalize, upsample ----
    pv = np.empty((BS, NC, NPATCH), dtype=np.float32)
    for i in range(NCORE):
        b, h = i // 2, i % 2
        # out[p, ci, q] -> rows ci*128+p of this (padded) half
        blk = np.asarray(res_b.results[i]["out"]).transpose(1, 0, 2)
        pv[b, h * CH:(h + 1) * CH] = blk.reshape(512, NPATCH)[:CH]
    z = pv.sum(axis=1, keepdims=True)           # sum_c lab = 1 per key
    lab_hat = (pv / z).reshape(BS, NC, GRID, GRID)
    out = np.broadcast_to(
        lab_hat[:, :, :, None, :, None],
        (BS, NC, GRID, P14, GRID, P14)).reshape(BS, NC, IMG, IMG)
    return out
